# revision 24
# baseline (speedup 1.0000x reference)
"""Trainium2 Bass kernel for multi-head attention (QKV proj + RoPE + softmax attention + out proj).

Problem: x[2,2048,2048], wq/wk/wv/wo[2048,2048], 16 heads x 128 dim, start_pos=0,
KV cache is fully overwritten before being read, so it never affects the output.

Sharding: 8 cores = 2 batches x 4 head-groups (4 heads each).  Each core computes
partial output  attn_heads(x[b]) @ woT[:, group]  and the host sums the 4 group
partials per batch.

v2 design (vs v1): all matmul operands in bf16 (fp32 PSUM accumulate; measured
end-to-end rel err ~5e-3 vs 2e-2 budget).  Q/K/V never round-trip through DRAM:
RoPE'd Q/K quadrants are placed into persistent SBUF tiles via SBUF->SBUF DMAs
(partition shifts), V is copied PSUM->SBUF directly.  Attention uses 1024-wide
t-chunks: QK pairs fill a [128,1024] PSUM tile (2 banks), one Exp activation
covers both halves (halves ACT overhead), PV accumulates into a [128,1024]
accumulator.  Softmax row-sums: adjacent exp-tile pairs summed on the Pool
engine, a bf16 tree on DVE reduces 8->1, and a single ones-matmul gives the
partition-broadcast row sums; rs is copied to SBUF so its PSUM slot frees
before the (slow) reciprocal.  The output projection for the first t-half is
interleaved at head boundaries of the second half (covers the pv-drain latency
and the ACT-bound qk/exp stretches); the rest runs as a pipelined tail.
"""

import math
import sys

sys.path.insert(0, "/opt/trn_rl_repo")

import numpy as np

import concourse.bacc as bacc
import concourse.mybir as mybir
import concourse.tile as tile
from concourse.bass_utils import run_bass_kernel_spmd

P = 128
F32 = mybir.dt.float32
BF16 = mybir.dt.bfloat16
MUL = mybir.AluOpType.mult
SUB = mybir.AluOpType.subtract
ADD = mybir.AluOpType.add


def build_attention_nc(T, E, HL, HD=128, CH=512):
    """One-core program: HL local heads, seq len T, embed E (full), head dim HD=128.

    Inputs (per core): xT[E,T], wqT/wkT[E,HL*HD] (pair-permuted), wvT[E,HL*HD],
    woT[HL*HD,E], cosF/sinF[P,T].  Output: outp[T,E] bf16 (partial, summed on host).
    """
    assert HD == P and E % P == 0 and T % P == 0 and T % CH == 0
    assert HL % 2 == 0 and HL * HD <= 512 and CH <= 512
    ET = E // P          # contraction tiles for the projections
    TC = T // CH         # t-chunks in phase A
    ST = T // P          # s-tiles
    D2 = HD // 2
    NPAIR = HL // 2
    CP = 1024            # attention t-chunk (2 PSUM banks wide)
    NCP = T // CP
    scale = 1.0 / math.sqrt(HD)

    nc = bacc.Bacc(None)
    xT = nc.dram_tensor("xT", [E, T], BF16, kind="ExternalInput")
    wqT = nc.dram_tensor("wqT", [E, HL * HD], BF16, kind="ExternalInput")
    wkT = nc.dram_tensor("wkT", [E, HL * HD], BF16, kind="ExternalInput")
    wvT = nc.dram_tensor("wvT", [E, HL * HD], BF16, kind="ExternalInput")
    woT = nc.dram_tensor("woT", [HL * HD, E], BF16, kind="ExternalInput")
    cosF = nc.dram_tensor("cosF", [P, T], F32, kind="ExternalInput")
    sinF = nc.dram_tensor("sinF", [P, T], F32, kind="ExternalInput")
    outp = nc.dram_tensor("outp", [T, E], BF16, kind="ExternalOutput")

    xT_t = xT.rearrange("(o p) t -> p o t", p=P)
    wq_t = wqT.rearrange("(o p) m -> p o m", p=P)
    wk_t = wkT.rearrange("(o p) m -> p o m", p=P)
    wv_t = wvT.rearrange("(o p) m -> p o m", p=P)
    wo_t = woT.rearrange("(h p) e -> p h e", p=P)

    with tile.TileContext(nc) as tc:
        with tc.tile_pool(name="keep", bufs=1) as keep:
            # persistent SBUF: Q^T/K^T per head [d, t], V [s, st, h*d], wo, attn
            qt_all = keep.tile([P, HL, T], BF16)
            kt_all = keep.tile([P, HL, T], BF16)
            v_all = keep.tile([P, ST, HL * HD], BF16)
            wo_sb = keep.tile([P, HL, E], BF16)
            attn0 = keep.tile([P, HL, CP], BF16)
            attn1 = keep.tile([P, HL, CP], BF16)
            attn_cp = [attn0, attn1]
            assert NCP == 2
            ones_sb = keep.tile([P, P], BF16)

            # ---------------- Phase A: QKV projections + RoPE ----------------
            with (
                tc.tile_pool(name="aw", bufs=1) as aw,
                tc.tile_pool(name="ax", bufs=2) as ax,
                tc.tile_pool(name="acs", bufs=2) as acs,
                tc.tile_pool(name="aps", bufs=2, space="PSUM") as aps,
                tc.tile_pool(name="arot", bufs=3) as arot,
            ):
                warm_f = aw.tile([P, CH], F32)
                nc.vector.memset(warm_f[:], 0.0)
                warm = aw.tile([P, CH], BF16)
                nc.vector.tensor_copy(warm[:], warm_f[:])
                ones_f32 = aw.tile([P, P], F32)
                nc.vector.memset(ones_f32[:], 1.0)
                nc.vector.tensor_copy(ones_sb[:], ones_f32[:])
                wq_sb = aw.tile([P, ET, HL * HD], BF16)
                wk_sb = aw.tile([P, ET, HL * HD], BF16)
                wv_sb = aw.tile([P, ET, HL * HD], BF16)
                x0_sb = ax.tile([P, ET, CH], BF16, tag="xc")
                # interleaved k-sliced loads: the k-th matmul of the first
                # accumulation only waits for its own slices.
                for k2 in range(0, ET, 2):
                    nc.sync.dma_start(wq_sb[:, k2:k2 + 2, :], wq_t[:, k2:k2 + 2, :])
                    nc.sync.dma_start(x0_sb[:, k2:k2 + 2, :], xT_t[:, k2:k2 + 2, 0:CH])
                for k2 in range(0, ET, 2):
                    nc.sync.dma_start(wk_sb[:, k2:k2 + 2, :], wk_t[:, k2:k2 + 2, :])
                for k2 in range(0, ET, 2):
                    nc.sync.dma_start(wv_sb[:, k2:k2 + 2, :], wv_t[:, k2:k2 + 2, :])
                for h in range(HL):
                    nc.gpsimd.dma_start(wo_sb[:, h, :], wo_t[:, h, :])

                with tc.tile_pool(name="wps", bufs=1, space="PSUM") as wps:
                    # p-state ramp + covers the initial weight/x DMA wait
                    wp = wps.tile([64, CH], F32)
                    for _ in range(13):
                        nc.tensor.matmul(wp[:], warm[:, 0:64], warm[:],
                                         start=True, stop=True)

                for c in range(TC):
                    csl = slice(c * CH, (c + 1) * CH)
                    if c == 0:
                        x_sb = x0_sb
                    else:
                        x_sb = ax.tile([P, ET, CH], BF16, tag="xc")
                        for k2 in range(0, ET, 4):
                            k3 = min(k2 + 4, ET)
                            nc.sync.dma_start(
                                x_sb[:, k2:k3, :], xT_t[:, k2:k3, csl])
                    cos_c = acs.tile([P, CH], F32, tag="cos")
                    sin_c = acs.tile([P, CH], F32, tag="sin")
                    nc.gpsimd.dma_start(cos_c[:], cosF[:, csl])
                    nc.gpsimd.dma_start(sin_c[:], sinF[:, csl])
                    for w_sb, dst in ((wq_sb, qt_all), (wk_sb, kt_all)):
                        for pr in range(NPAIR):
                            mA, mB = 2 * pr, 2 * pr + 1
                            psA = aps.tile([P, CH], F32, tag="psA")
                            psB = aps.tile([P, CH], F32, tag="psB")
                            for k in range(ET):
                                nc.tensor.matmul(
                                    psA[:], w_sb[:, k, mA * P:(mA + 1) * P],
                                    x_sb[:, k, :], start=(k == 0), stop=(k == ET - 1))
                            for k in range(ET):
                                nc.tensor.matmul(
                                    psB[:], w_sb[:, k, mB * P:(mB + 1) * P],
                                    x_sb[:, k, :], start=(k == 0), stop=(k == ET - 1))
                            # rope: psA rows = [hA even-dims | hB even-dims],
                            # psB rows = [hA odd | hB odd]
                            t1 = arot.tile([P, CH], F32, tag="t1")
                            t2 = arot.tile([P, CH], F32, tag="t2")
                            rA = arot.tile([P, CH], BF16, tag="rA")
                            nc.vector.tensor_tensor(t1[:], psA[:], cos_c[:], MUL)
                            nc.vector.tensor_tensor(t2[:], psB[:], sin_c[:], MUL)
                            nc.vector.tensor_tensor(rA[:], t1[:], t2[:], SUB)
                            t3 = arot.tile([P, CH], F32, tag="t1")
                            t4 = arot.tile([P, CH], F32, tag="t2")
                            rB = arot.tile([P, CH], BF16, tag="rA")
                            nc.vector.tensor_tensor(t3[:], psA[:], sin_c[:], MUL)
                            nc.vector.tensor_tensor(t4[:], psB[:], cos_c[:], MUL)
                            nc.vector.tensor_tensor(rB[:], t3[:], t4[:], ADD)
                            ha, hb = 2 * pr, 2 * pr + 1
                            # quadrant placement (partition shifts via DMA):
                            nc.gpsimd.dma_start(dst[0:D2, ha, csl], rA[0:D2, :])
                            nc.gpsimd.dma_start(dst[0:D2, hb, csl], rA[D2:P, :])
                            nc.gpsimd.dma_start(dst[D2:P, ha, csl], rB[0:D2, :])
                            nc.gpsimd.dma_start(dst[D2:P, hb, csl], rB[D2:P, :])
                    for st in range(CH // P):
                        psV = aps.tile([P, HL * HD], F32, tag="psV")
                        for k in range(ET):
                            nc.tensor.matmul(
                                psV[:], x_sb[:, k, st * P:(st + 1) * P],
                                wv_sb[:, k, :], start=(k == 0), stop=(k == ET - 1))
                        nc.scalar.copy(v_all[:, c * (CH // P) + st, :], psV[:])

            # ---------------- Phase B: attention + interleaved out-proj ----------------
            # C-groups: out[t_tile, oc] = sum_h attn[d, t_tile]^T @ wo[d, oc].
            # Output rows are batched: 4 oc-groups fill one [P, E] bf16 row
            # tile, then a single wide DMA (4KB lines) writes the row.
            c_rows = {}

            def emit_c_group(cp, tt, oc, ops_pool, row_pool, idx):
                tsl = slice(cp * CP + tt * P, cp * CP + (tt + 1) * P)
                osl = slice(oc * CH, (oc + 1) * CH)
                ops = ops_pool.tile([P, CP], F32, tag="sps")
                for hh in range(HL):
                    nc.tensor.matmul(
                        ops[:, 0:CH], attn_cp[cp][:, hh, tt * P:(tt + 1) * P],
                        wo_sb[:, hh, osl], start=(hh == 0), stop=(hh == HL - 1))
                if oc == 0:
                    c_rows[(cp, tt)] = row_pool.tile(
                        [P, E], BF16, tag="orow", name=f"orow_{cp}_{tt}")
                orow = c_rows[(cp, tt)]
                if idx % 2 == 0 or idx < 24:
                    nc.vector.tensor_copy(orow[:, osl], ops[:, 0:CH])
                else:
                    nc.scalar.copy(orow[:, osl], ops[:, 0:CH])
                if oc == E // CH - 1:
                    if idx < 24:
                        eng = nc.sync if (tt % 2 == 0) else nc.gpsimd
                    else:  # tail: spread the big row writes over 3 queues
                        eng = (nc.sync, nc.gpsimd,
                               nc.scalar)[(cp * 8 + tt) % 3]
                    eng.dma_start(outp[tsl, :], orow[:])
                    del c_rows[(cp, tt)]

            c_groups = [(cp, tt, oc) for cp in range(NCP)
                        for tt in range(CP // P) for oc in range(E // CH)]
            c_next = 0
            with (
                tc.tile_pool(name="bsp", bufs=2, space="PSUM") as bsp,
                tc.tile_pool(name="bpv", bufs=1, space="PSUM") as bpv,
                tc.tile_pool(name="brsp", bufs=1, space="PSUM") as brsp,
                tc.tile_pool(name="bpt", bufs=6) as bpt,
                tc.tile_pool(name="bpair", bufs=4) as bpair,
                tc.tile_pool(name="brc", bufs=2) as brc,
                tc.tile_pool(name="bco", bufs=3) as bco,
            ):
                for cp in range(NCP):
                    for h in range(HL):
                        hsl = slice(h * HD, (h + 1) * HD)
                        pv = bpv.tile([P, CP], F32)
                        rs = brsp.tile([P, CP], F32)
                        prev_pt = None
                        pairs, quads = [], []
                        rs_pending = []  # (piece, ready_st); emit 2 sts later
                        rs_emitted = 0

                        def emit_rs(piece, is_last):
                            nonlocal rs_emitted
                            nc.tensor.matmul(
                                rs[:, 0:CH], ones_sb[:], piece[:, 0:CH],
                                start=(rs_emitted == 0), stop=is_last)
                            nc.tensor.matmul(
                                rs[:, CH:CP], ones_sb[:], piece[:, CH:CP],
                                start=(rs_emitted == 0), stop=is_last)
                            rs_emitted += 1
                        for st in range(ST):
                            sps = bsp.tile([P, CP], F32, tag="sps")
                            ksl = kt_all[:, h, st * P:(st + 1) * P]
                            nc.tensor.matmul(
                                sps[:, 0:CH], ksl,
                                qt_all[:, h, cp * CP:cp * CP + CH],
                                start=True, stop=True)
                            nc.tensor.matmul(
                                sps[:, CH:CP], ksl,
                                qt_all[:, h, cp * CP + CH:(cp + 1) * CP],
                                start=True, stop=True)
                            pt = bpt.tile([P, CP], BF16, tag="pt")
                            nc.scalar.activation(
                                pt[:], sps[:], mybir.ActivationFunctionType.Exp,
                                scale=scale)
                            vsl = v_all[:, st, hsl]
                            nc.tensor.matmul(
                                pv[:, 0:CH], vsl, pt[:, 0:CH],
                                start=(st == 0), stop=(st == ST - 1))
                            nc.tensor.matmul(
                                pv[:, CH:CP], vsl, pt[:, CH:CP],
                                start=(st == 0), stop=(st == ST - 1))
                            while rs_pending and rs_pending[0][1] + 2 <= st:
                                emit_rs(rs_pending.pop(0)[0], False)
                            # Graded DVE reduction: sts 0-7 reduce to an oct,
                            # 8-11 to a quad, 12-13 / 14-15 stay pairs.  Each
                            # finished piece feeds one ones-matmul, so the
                            # last row-sum matmul only trails the final
                            # pair-add (short chain), yet rs needs just 4
                            # accumulation steps per head.
                            if st % 2 == 0:
                                prev_pt = pt
                                continue
                            p2 = bpair.tile([P, CP], BF16, tag="p2", bufs=4)
                            nc.vector.tensor_tensor(
                                p2[:], prev_pt[:], pt[:], ADD)
                            piece = None
                            if st in (1, 5, 9):
                                pairs.append(p2)
                            elif st in (3, 7, 11):
                                q4 = bpair.tile([P, CP], BF16, tag="q4", bufs=3)
                                nc.vector.tensor_tensor(
                                    q4[:], pairs.pop()[:], p2[:], ADD)
                                if st == 11:
                                    piece = q4   # sts 8-11 reduced to a quad
                                else:
                                    quads.append(q4)
                                if st == 7:
                                    q8 = bpair.tile([P, CP], BF16, tag="q8",
                                                    bufs=2)
                                    nc.vector.tensor_tensor(
                                        q8[:], quads[0][:], quads[1][:], ADD)
                                    quads = []
                                    piece = q8   # sts 0-7 reduced to one oct
                            else:
                                piece = p2       # sts 12-13, 14-15: raw pair
                            if piece is not None:
                                rs_pending.append((piece, st))
                        # cover the final rs matmuls / recip with out-proj work
                        if cp > 0:
                            for _ in range(2):
                                gcp, gtt, goc = c_groups[c_next]
                                emit_c_group(gcp, gtt, goc, bsp, bco, c_next)
                                c_next += 1
                        while rs_pending:
                            piece, _ = rs_pending.pop(0)
                            emit_rs(piece, not rs_pending)
                        rec = brc.tile([P, CP], F32, tag="rec")
                        scr8 = brc.tile([P, CP], F32, tag="scr")
                        nc.vector.reciprocal_approx_accurate(
                            out=rec[:], in_=rs[:], scratch=scr8[:])
                        nc.vector.tensor_tensor(
                            attn_cp[cp][:, h, :], pv[:], rec[:], MUL)
                        # interleave first-cp out-proj at cp1 head boundaries
                        if cp > 0:
                            for _ in range(4):
                                gcp, gtt, goc = c_groups[c_next]
                                emit_c_group(gcp, gtt, goc, bsp, bco, c_next)
                                c_next += 1

            # ---------------- Phase C tail: remaining out-proj groups ----------------
            with (
                tc.tile_pool(name="cout", bufs=3) as cout,
                tc.tile_pool(name="cps", bufs=3, space="PSUM") as cps,
            ):
                while c_next < len(c_groups):
                    gcp, gtt, goc = c_groups[c_next]
                    emit_c_group(gcp, gtt, goc, cps, cout, c_next)
                    c_next += 1

    nc.finalize()
    return nc


# ---------------------------------------------------------------------------
# Host-side wrapper
# ---------------------------------------------------------------------------

_B, _T, _EMB = 2, 2048, 2048
_HQ, _HD = 16, 128
_GROUPS = 4                      # head groups; 2 batches x 4 groups = 8 cores
_HL = _HQ // _GROUPS             # 4 local heads per core

_nc_cache = {}


def _get_nc():
    key = (_T, _EMB, _HL, _HD)
    if key not in _nc_cache:
        _nc_cache[key] = build_attention_nc(_T, _EMB, _HL, _HD, CH=512)
    return _nc_cache[key]


def _prep_core_inputs(x, wq, wk, wv, wo, fc, fs):
    """Per-core input dicts for 8 cores (core = 4*batch + group)."""
    import ml_dtypes
    bf16 = ml_dtypes.bfloat16
    # RoPE pair-permutation within each head: [even dims, odd dims]
    perm = np.concatenate([np.arange(0, _HD, 2), np.arange(1, _HD, 2)])
    cosF = np.ascontiguousarray(np.tile(fc.T, (2, 1)), dtype=np.float32)  # [128,T]
    sinF = np.ascontiguousarray(np.tile(fs.T, (2, 1)), dtype=np.float32)

    xT = [np.ascontiguousarray(x[b].T).astype(bf16) for b in range(_B)]

    in_maps = []
    for b in range(_B):
        for g in range(_GROUPS):
            heads = [g * _HL + h for h in range(_HL)]
            # device q/k row order: per pair (h0,h1): [h0_e, h1_e], [h0_o, h1_o]
            rows = []
            for pr in range(_HL // 2):
                h0, h1 = heads[2 * pr], heads[2 * pr + 1]
                for half in (perm[:64], perm[64:]):
                    rows.append(h0 * _HD + half)
                    rows.append(h1 * _HD + half)
            rows = np.concatenate(rows)
            vrows = np.concatenate([np.arange(h * _HD, (h + 1) * _HD) for h in heads])
            in_maps.append({
                "xT": xT[b],
                "wqT": np.ascontiguousarray(wq[rows].T).astype(bf16),
                "wkT": np.ascontiguousarray(wk[rows].T).astype(bf16),
                "wvT": np.ascontiguousarray(wv[vrows].T).astype(bf16),
                "woT": np.ascontiguousarray(wo[:, vrows].T).astype(bf16),
                "cosF": cosF,
                "sinF": sinF,
            })
    return in_maps


def kernel(**inputs):
    x = np.asarray(inputs["x"], dtype=np.float32)
    wq = np.asarray(inputs["wq"], dtype=np.float32)
    wk = np.asarray(inputs["wk"], dtype=np.float32)
    wv = np.asarray(inputs["wv"], dtype=np.float32)
    wo = np.asarray(inputs["wo"], dtype=np.float32)
    fc = np.asarray(inputs["freqs_cos"], dtype=np.float32)
    fs = np.asarray(inputs["freqs_sin"], dtype=np.float32)
    # start_pos is 0 (cache region [0, T) is fully overwritten before the read,
    # so k_cache/v_cache never contribute to the output).

    nc = _get_nc()
    in_maps = _prep_core_inputs(x, wq, wk, wv, wo, fc, fs)
    res = run_bass_kernel_spmd(nc, in_maps, core_ids=list(range(8)))

    out = np.empty((_B, _T, _EMB), dtype=np.float32)
    for b in range(_B):
        acc = np.zeros((_T, _EMB), dtype=np.float32)
        for g in range(_GROUPS):
            acc += res.results[4 * b + g]["outp"].astype(np.float32)
        out[b] = acc
    return out


# revision 25
# speedup vs baseline: 1.1983x; 1.1983x over previous
"""Trainium2 Bass kernel for multi-head attention (QKV proj + RoPE + softmax attention + out proj).

Problem: x[2,2048,2048], wq/wk/wv/wo[2048,2048], 16 heads x 128 dim, start_pos=0,
KV cache is fully overwritten before being read, so it never affects the output.

Sharding: 8 cores = 2 batches x 4 head-groups (4 heads each).  Each core computes
partial output  attn_heads(x[b]) @ woT[:, group]  and the host sums the 4 group
partials per batch.

v2 design (vs v1): all matmul operands in bf16 (fp32 PSUM accumulate; measured
end-to-end rel err ~5e-3 vs 2e-2 budget).  Q/K/V never round-trip through DRAM:
RoPE'd Q/K quadrants are placed into persistent SBUF tiles via SBUF->SBUF DMAs
(partition shifts), V is copied PSUM->SBUF directly.  Attention uses 1024-wide
t-chunks: QK pairs fill a [128,1024] PSUM tile (2 banks), one Exp activation
covers both halves (halves ACT overhead), PV accumulates into a [128,1024]
accumulator.  Softmax row-sums: adjacent exp-tile pairs summed on the Pool
engine, a bf16 tree on DVE reduces 8->1, and a single ones-matmul gives the
partition-broadcast row sums; rs is copied to SBUF so its PSUM slot frees
before the (slow) reciprocal.  The output projection for the first t-half is
interleaved at head boundaries of the second half (covers the pv-drain latency
and the ACT-bound qk/exp stretches); the rest runs as a pipelined tail.
"""

import math
import sys

sys.path.insert(0, "/opt/trn_rl_repo")

import numpy as np

import concourse.bacc as bacc
import concourse.mybir as mybir
import concourse.tile as tile
from concourse.bass_utils import run_bass_kernel_spmd

P = 128
F32 = mybir.dt.float32
BF16 = mybir.dt.bfloat16
MUL = mybir.AluOpType.mult
SUB = mybir.AluOpType.subtract
ADD = mybir.AluOpType.add


def build_attention_nc(T, E, HL, HD=128, CH=512):
    """One-core program: HL local heads, seq len T, embed E (full), head dim HD=128.

    Inputs (per core): xT[E,T], wqT/wkT[E,HL*HD] (pair-permuted), wvT[E,HL*HD],
    woT[HL*HD,E], cosF/sinF[P,T].  Output: outp[T,E] bf16 (partial, summed on host).
    """
    assert HD == P and E % P == 0 and T % P == 0 and T % CH == 0
    assert HL % 2 == 0 and HL * HD <= 512 and CH <= 512
    ET = E // P          # contraction tiles for the projections
    TC = T // CH         # t-chunks in phase A
    ST = T // P          # s-tiles
    D2 = HD // 2
    NPAIR = HL // 2
    CP = 1024            # attention t-chunk (2 PSUM banks wide)
    NCP = T // CP
    scale = 1.0 / math.sqrt(HD)

    nc = bacc.Bacc(None)
    xT = nc.dram_tensor("xT", [E, T], BF16, kind="ExternalInput")
    wqT = nc.dram_tensor("wqT", [E, HL * HD], BF16, kind="ExternalInput")
    wkT = nc.dram_tensor("wkT", [E, HL * HD], BF16, kind="ExternalInput")
    wvT = nc.dram_tensor("wvT", [E, HL * HD], BF16, kind="ExternalInput")
    woT = nc.dram_tensor("woT", [HL * HD, E], BF16, kind="ExternalInput")
    cosF = nc.dram_tensor("cosF", [P, T], F32, kind="ExternalInput")
    sinF = nc.dram_tensor("sinF", [P, T], F32, kind="ExternalInput")
    outp = nc.dram_tensor("outp", [T, E], BF16, kind="ExternalOutput")

    xT_t = xT.rearrange("(o p) t -> p o t", p=P)
    wq_t = wqT.rearrange("(o p) m -> p o m", p=P)
    wk_t = wkT.rearrange("(o p) m -> p o m", p=P)
    wv_t = wvT.rearrange("(o p) m -> p o m", p=P)
    wo_t = woT.rearrange("(h p) e -> p h e", p=P)

    with tile.TileContext(nc) as tc:
        with tc.tile_pool(name="keep", bufs=1) as keep:
            # persistent SBUF: Q^T/K^T per head [d, t], V [s, st, h*d], wo, attn
            qt_all = keep.tile([P, HL, T], BF16)
            kt_all = keep.tile([P, HL, T], BF16)
            v_all = keep.tile([P, ST, HL * HD], BF16)
            wo_sb = keep.tile([P, HL, E], BF16)
            attn0 = keep.tile([P, HL, CP], BF16)
            attn1 = keep.tile([P, HL, CP], BF16)
            attn_cp = [attn0, attn1]
            assert NCP == 2
            ones_sb = keep.tile([P, P], BF16)

            # ---------------- Phase A: QKV projections + RoPE ----------------
            with (
                tc.tile_pool(name="aw", bufs=1) as aw,
                tc.tile_pool(name="ax", bufs=2) as ax,
                tc.tile_pool(name="acs", bufs=2) as acs,
                tc.tile_pool(name="aps", bufs=2, space="PSUM") as aps,
                tc.tile_pool(name="arot", bufs=3) as arot,
            ):
                warm_f = aw.tile([P, CH], F32)
                nc.vector.memset(warm_f[:], 0.0)
                warm = aw.tile([P, CH], BF16)
                nc.vector.tensor_copy(warm[:], warm_f[:])
                ones_f32 = aw.tile([P, P], F32)
                nc.vector.memset(ones_f32[:], 1.0)
                nc.vector.tensor_copy(ones_sb[:], ones_f32[:])
                wq_sb = aw.tile([P, ET, HL * HD], BF16)
                wk_sb = aw.tile([P, ET, HL * HD], BF16)
                wv_sb = aw.tile([P, ET, HL * HD], BF16)
                x0_sb = ax.tile([P, ET, CH], BF16, tag="xc")
                # interleaved k-sliced loads: the k-th matmul of the first
                # accumulation only waits for its own slices.
                for k2 in range(0, ET, 2):
                    nc.sync.dma_start(wq_sb[:, k2:k2 + 2, :], wq_t[:, k2:k2 + 2, :])
                    nc.sync.dma_start(x0_sb[:, k2:k2 + 2, :], xT_t[:, k2:k2 + 2, 0:CH])
                for k2 in range(0, ET, 2):
                    nc.sync.dma_start(wk_sb[:, k2:k2 + 2, :], wk_t[:, k2:k2 + 2, :])
                for k2 in range(0, ET, 2):
                    nc.sync.dma_start(wv_sb[:, k2:k2 + 2, :], wv_t[:, k2:k2 + 2, :])
                for h in range(HL):
                    nc.gpsimd.dma_start(wo_sb[:, h, :], wo_t[:, h, :])

                with tc.tile_pool(name="wps", bufs=1, space="PSUM") as wps:
                    # p-state ramp + covers the initial weight/x DMA wait
                    wp = wps.tile([64, CH], F32)
                    for _ in range(13):
                        nc.tensor.matmul(wp[:], warm[:, 0:64], warm[:],
                                         start=True, stop=True)

                for c in range(TC):
                    csl = slice(c * CH, (c + 1) * CH)
                    if c == 0:
                        x_sb = x0_sb
                    else:
                        x_sb = ax.tile([P, ET, CH], BF16, tag="xc")
                        for k2 in range(0, ET, 4):
                            k3 = min(k2 + 4, ET)
                            nc.sync.dma_start(
                                x_sb[:, k2:k3, :], xT_t[:, k2:k3, csl])
                    cos_c = acs.tile([P, CH], F32, tag="cos")
                    sin_c = acs.tile([P, CH], F32, tag="sin")
                    nc.gpsimd.dma_start(cos_c[:], cosF[:, csl])
                    nc.gpsimd.dma_start(sin_c[:], sinF[:, csl])
                    for w_sb, dst in ((wq_sb, qt_all), (wk_sb, kt_all)):
                        for pr in range(NPAIR):
                            mA, mB = 2 * pr, 2 * pr + 1
                            psA = aps.tile([P, CH], F32, tag="psA")
                            psB = aps.tile([P, CH], F32, tag="psB")
                            for k in range(ET):
                                nc.tensor.matmul(
                                    psA[:], w_sb[:, k, mA * P:(mA + 1) * P],
                                    x_sb[:, k, :], start=(k == 0), stop=(k == ET - 1))
                            for k in range(ET):
                                nc.tensor.matmul(
                                    psB[:], w_sb[:, k, mB * P:(mB + 1) * P],
                                    x_sb[:, k, :], start=(k == 0), stop=(k == ET - 1))
                            # rope: psA rows = [hA even-dims | hB even-dims],
                            # psB rows = [hA odd | hB odd]
                            t1 = arot.tile([P, CH], F32, tag="t1")
                            t2 = arot.tile([P, CH], F32, tag="t2")
                            rA = arot.tile([P, CH], BF16, tag="rA")
                            nc.vector.tensor_tensor(t1[:], psA[:], cos_c[:], MUL)
                            nc.vector.tensor_tensor(t2[:], psB[:], sin_c[:], MUL)
                            nc.vector.tensor_tensor(rA[:], t1[:], t2[:], SUB)
                            t3 = arot.tile([P, CH], F32, tag="t1")
                            t4 = arot.tile([P, CH], F32, tag="t2")
                            rB = arot.tile([P, CH], BF16, tag="rA")
                            nc.vector.tensor_tensor(t3[:], psA[:], sin_c[:], MUL)
                            nc.vector.tensor_tensor(t4[:], psB[:], cos_c[:], MUL)
                            nc.vector.tensor_tensor(rB[:], t3[:], t4[:], ADD)
                            ha, hb = 2 * pr, 2 * pr + 1
                            # quadrant placement (partition shifts via DMA):
                            nc.gpsimd.dma_start(dst[0:D2, ha, csl], rA[0:D2, :])
                            nc.gpsimd.dma_start(dst[0:D2, hb, csl], rA[D2:P, :])
                            nc.gpsimd.dma_start(dst[D2:P, ha, csl], rB[0:D2, :])
                            nc.gpsimd.dma_start(dst[D2:P, hb, csl], rB[D2:P, :])
                    for st in range(CH // P):
                        psV = aps.tile([P, HL * HD], F32, tag="psV")
                        for k in range(ET):
                            nc.tensor.matmul(
                                psV[:], x_sb[:, k, st * P:(st + 1) * P],
                                wv_sb[:, k, :], start=(k == 0), stop=(k == ET - 1))
                        nc.scalar.copy(v_all[:, c * (CH // P) + st, :], psV[:])

            # ---------------- Phase B: attention + interleaved out-proj ----------------
            # C-groups: out[t_tile, oc] = sum_h attn[d, t_tile]^T @ wo[d, oc].
            # Output rows are batched: 4 oc-groups fill one [P, E] bf16 row
            # tile, then a single wide DMA (4KB lines) writes the row.
            c_rows = {}

            def emit_c_group(cp, tt, oc, ops_pool, row_pool, idx):
                tsl = slice(cp * CP + tt * P, cp * CP + (tt + 1) * P)
                osl = slice(oc * CH, (oc + 1) * CH)
                ops = ops_pool.tile([P, CP], F32, tag="sps")
                for hh in range(HL):
                    nc.tensor.matmul(
                        ops[:, 0:CH], attn_cp[cp][:, hh, tt * P:(tt + 1) * P],
                        wo_sb[:, hh, osl], start=(hh == 0), stop=(hh == HL - 1))
                if oc == 0:
                    c_rows[(cp, tt)] = row_pool.tile(
                        [P, E], BF16, tag="orow", name=f"orow_{cp}_{tt}")
                orow = c_rows[(cp, tt)]
                if idx % 2 == 0 or idx < 24:
                    nc.vector.tensor_copy(orow[:, osl], ops[:, 0:CH])
                else:
                    nc.scalar.copy(orow[:, osl], ops[:, 0:CH])
                if oc == E // CH - 1:
                    if idx < 24:
                        eng = nc.sync if (tt % 2 == 0) else nc.gpsimd
                    else:  # tail: spread the big row writes over 3 queues
                        eng = (nc.sync, nc.gpsimd,
                               nc.scalar)[(cp * 8 + tt) % 3]
                    eng.dma_start(outp[tsl, :], orow[:])
                    del c_rows[(cp, tt)]

            c_groups = [(cp, tt, oc) for cp in range(NCP)
                        for tt in range(CP // P) for oc in range(E // CH)]
            c_next = 0
            with (
                tc.tile_pool(name="bsp", bufs=2, space="PSUM") as bsp,
                tc.tile_pool(name="bpv", bufs=1, space="PSUM") as bpv,
                tc.tile_pool(name="brsp", bufs=1, space="PSUM") as brsp,
                tc.tile_pool(name="bpt", bufs=6) as bpt,
                tc.tile_pool(name="bpair", bufs=4) as bpair,
                tc.tile_pool(name="brc", bufs=2) as brc,
                tc.tile_pool(name="bco", bufs=3) as bco,
            ):
                for cp in range(NCP):
                    for h in range(HL):
                        hsl = slice(h * HD, (h + 1) * HD)
                        pv = bpv.tile([P, CP], F32)
                        rs = brsp.tile([P, CP], F32)
                        prev_pt = None
                        pairs, quads = [], []
                        rs_pending = []  # (piece, ready_st); emit 2 sts later
                        rs_emitted = 0

                        def emit_rs(piece, is_last):
                            nonlocal rs_emitted
                            nc.tensor.matmul(
                                rs[:, 0:CH], ones_sb[:], piece[:, 0:CH],
                                start=(rs_emitted == 0), stop=is_last)
                            nc.tensor.matmul(
                                rs[:, CH:CP], ones_sb[:], piece[:, CH:CP],
                                start=(rs_emitted == 0), stop=is_last)
                            rs_emitted += 1
                        for st in range(ST):
                            sps = bsp.tile([P, CP], F32, tag="sps")
                            ksl = kt_all[:, h, st * P:(st + 1) * P]
                            nc.tensor.matmul(
                                sps[:, 0:CH], ksl,
                                qt_all[:, h, cp * CP:cp * CP + CH],
                                start=True, stop=True)
                            nc.tensor.matmul(
                                sps[:, CH:CP], ksl,
                                qt_all[:, h, cp * CP + CH:(cp + 1) * CP],
                                start=True, stop=True)
                            pt = bpt.tile([P, CP], BF16, tag="pt")
                            nc.scalar.activation(
                                pt[:], sps[:], mybir.ActivationFunctionType.Exp,
                                scale=scale)
                            vsl = v_all[:, st, hsl]
                            nc.tensor.matmul(
                                pv[:, 0:CH], vsl, pt[:, 0:CH],
                                start=(st == 0), stop=(st == ST - 1))
                            nc.tensor.matmul(
                                pv[:, CH:CP], vsl, pt[:, CH:CP],
                                start=(st == 0), stop=(st == ST - 1))
                            while rs_pending and rs_pending[0][1] + 2 <= st:
                                emit_rs(rs_pending.pop(0)[0], False)
                            # Graded DVE reduction: sts 0-7 reduce to an oct,
                            # 8-11 to a quad, 12-13 / 14-15 stay pairs.  Each
                            # finished piece feeds one ones-matmul, so the
                            # last row-sum matmul only trails the final
                            # pair-add (short chain), yet rs needs just 4
                            # accumulation steps per head.
                            if st % 2 == 0:
                                prev_pt = pt
                                continue
                            p2 = bpair.tile([P, CP], BF16, tag="p2", bufs=4)
                            nc.vector.tensor_tensor(
                                p2[:], prev_pt[:], pt[:], ADD)
                            piece = None
                            if st in (1, 5, 9):
                                pairs.append(p2)
                            elif st in (3, 7, 11):
                                q4 = bpair.tile([P, CP], BF16, tag="q4", bufs=3)
                                nc.vector.tensor_tensor(
                                    q4[:], pairs.pop()[:], p2[:], ADD)
                                if st == 11:
                                    piece = q4   # sts 8-11 reduced to a quad
                                else:
                                    quads.append(q4)
                                if st == 7:
                                    q8 = bpair.tile([P, CP], BF16, tag="q8",
                                                    bufs=2)
                                    nc.vector.tensor_tensor(
                                        q8[:], quads[0][:], quads[1][:], ADD)
                                    quads = []
                                    piece = q8   # sts 0-7 reduced to one oct
                            else:
                                piece = p2       # sts 12-13, 14-15: raw pair
                            if piece is not None:
                                rs_pending.append((piece, st))
                        # drain pv to SBUF immediately (releases its PSUM banks
                        # for the next head); normalize from the copy once the
                        # reciprocal lands — off the critical path.
                        attn_un = brc.tile([P, CP], BF16, tag="un")
                        nc.vector.tensor_copy(attn_un[:], pv[:])
                        # cover the final rs matmuls / recip with out-proj work
                        if cp > 0:
                            for _ in range(2):
                                gcp, gtt, goc = c_groups[c_next]
                                emit_c_group(gcp, gtt, goc, bsp, bco, c_next)
                                c_next += 1
                        while rs_pending:
                            piece, _ = rs_pending.pop(0)
                            emit_rs(piece, not rs_pending)
                        rec = brc.tile([P, CP], F32, tag="rec")
                        scr8 = brc.tile([P, CP], F32, tag="scr")
                        nc.vector.reciprocal_approx_accurate(
                            out=rec[:], in_=rs[:], scratch=scr8[:])
                        nc.vector.tensor_tensor(
                            attn_cp[cp][:, h, :], attn_un[:], rec[:], MUL)
                        # interleave first-cp out-proj at cp1 head boundaries
                        if cp > 0:
                            for _ in range(4):
                                gcp, gtt, goc = c_groups[c_next]
                                emit_c_group(gcp, gtt, goc, bsp, bco, c_next)
                                c_next += 1

            # ---------------- Phase C tail: remaining out-proj groups ----------------
            with (
                tc.tile_pool(name="cout", bufs=3) as cout,
                tc.tile_pool(name="cps", bufs=3, space="PSUM") as cps,
            ):
                while c_next < len(c_groups):
                    gcp, gtt, goc = c_groups[c_next]
                    emit_c_group(gcp, gtt, goc, cps, cout, c_next)
                    c_next += 1

    nc.finalize()
    return nc


# ---------------------------------------------------------------------------
# Host-side wrapper
# ---------------------------------------------------------------------------

_B, _T, _EMB = 2, 2048, 2048
_HQ, _HD = 16, 128
_GROUPS = 4                      # head groups; 2 batches x 4 groups = 8 cores
_HL = _HQ // _GROUPS             # 4 local heads per core

_nc_cache = {}


def _get_nc():
    key = (_T, _EMB, _HL, _HD)
    if key not in _nc_cache:
        _nc_cache[key] = build_attention_nc(_T, _EMB, _HL, _HD, CH=512)
    return _nc_cache[key]


def _prep_core_inputs(x, wq, wk, wv, wo, fc, fs):
    """Per-core input dicts for 8 cores (core = 4*batch + group)."""
    import ml_dtypes
    bf16 = ml_dtypes.bfloat16
    # RoPE pair-permutation within each head: [even dims, odd dims]
    perm = np.concatenate([np.arange(0, _HD, 2), np.arange(1, _HD, 2)])
    cosF = np.ascontiguousarray(np.tile(fc.T, (2, 1)), dtype=np.float32)  # [128,T]
    sinF = np.ascontiguousarray(np.tile(fs.T, (2, 1)), dtype=np.float32)

    xT = [np.ascontiguousarray(x[b].T).astype(bf16) for b in range(_B)]

    in_maps = []
    for b in range(_B):
        for g in range(_GROUPS):
            heads = [g * _HL + h for h in range(_HL)]
            # device q/k row order: per pair (h0,h1): [h0_e, h1_e], [h0_o, h1_o]
            rows = []
            for pr in range(_HL // 2):
                h0, h1 = heads[2 * pr], heads[2 * pr + 1]
                for half in (perm[:64], perm[64:]):
                    rows.append(h0 * _HD + half)
                    rows.append(h1 * _HD + half)
            rows = np.concatenate(rows)
            vrows = np.concatenate([np.arange(h * _HD, (h + 1) * _HD) for h in heads])
            in_maps.append({
                "xT": xT[b],
                "wqT": np.ascontiguousarray(wq[rows].T).astype(bf16),
                "wkT": np.ascontiguousarray(wk[rows].T).astype(bf16),
                "wvT": np.ascontiguousarray(wv[vrows].T).astype(bf16),
                "woT": np.ascontiguousarray(wo[:, vrows].T).astype(bf16),
                "cosF": cosF,
                "sinF": sinF,
            })
    return in_maps


def kernel(**inputs):
    x = np.asarray(inputs["x"], dtype=np.float32)
    wq = np.asarray(inputs["wq"], dtype=np.float32)
    wk = np.asarray(inputs["wk"], dtype=np.float32)
    wv = np.asarray(inputs["wv"], dtype=np.float32)
    wo = np.asarray(inputs["wo"], dtype=np.float32)
    fc = np.asarray(inputs["freqs_cos"], dtype=np.float32)
    fs = np.asarray(inputs["freqs_sin"], dtype=np.float32)
    # start_pos is 0 (cache region [0, T) is fully overwritten before the read,
    # so k_cache/v_cache never contribute to the output).

    nc = _get_nc()
    in_maps = _prep_core_inputs(x, wq, wk, wv, wo, fc, fs)
    res = run_bass_kernel_spmd(nc, in_maps, core_ids=list(range(8)))

    out = np.empty((_B, _T, _EMB), dtype=np.float32)
    for b in range(_B):
        acc = np.zeros((_T, _EMB), dtype=np.float32)
        for g in range(_GROUPS):
            acc += res.results[4 * b + g]["outp"].astype(np.float32)
        out[b] = acc
    return out


# revision 28
# speedup vs baseline: 1.2477x; 1.0412x over previous
"""Trainium2 Bass kernel for multi-head attention (QKV proj + RoPE + softmax attention + out proj).

Problem: x[2,2048,2048], wq/wk/wv/wo[2048,2048], 16 heads x 128 dim, start_pos=0,
KV cache is fully overwritten before being read, so it never affects the output.

Sharding: 8 cores = 2 batches x 4 head-groups (4 heads each).  Each core computes
partial output  attn_heads(x[b]) @ woT[:, group]  and the host sums the 4 group
partials per batch.

v2 design (vs v1): all matmul operands in bf16 (fp32 PSUM accumulate; measured
end-to-end rel err ~5e-3 vs 2e-2 budget).  Q/K/V never round-trip through DRAM:
RoPE'd Q/K quadrants are placed into persistent SBUF tiles via SBUF->SBUF DMAs
(partition shifts), V is copied PSUM->SBUF directly.  Attention uses 1024-wide
t-chunks: QK pairs fill a [128,1024] PSUM tile (2 banks), one Exp activation
covers both halves (halves ACT overhead), PV accumulates into a [128,1024]
accumulator.  Softmax row-sums: adjacent exp-tile pairs summed on the Pool
engine, a bf16 tree on DVE reduces 8->1, and a single ones-matmul gives the
partition-broadcast row sums; rs is copied to SBUF so its PSUM slot frees
before the (slow) reciprocal.  The output projection for the first t-half is
interleaved at head boundaries of the second half (covers the pv-drain latency
and the ACT-bound qk/exp stretches); the rest runs as a pipelined tail.
"""

import math
import sys

sys.path.insert(0, "/opt/trn_rl_repo")

import numpy as np

import concourse.bacc as bacc
import concourse.mybir as mybir
import concourse.tile as tile
from concourse.bass_utils import run_bass_kernel_spmd

P = 128
F32 = mybir.dt.float32
BF16 = mybir.dt.bfloat16
MUL = mybir.AluOpType.mult
SUB = mybir.AluOpType.subtract
ADD = mybir.AluOpType.add


def build_attention_nc(T, E, HL, HD=128, CH=512):
    """One-core program: HL local heads, seq len T, embed E (full), head dim HD=128.

    Inputs (per core): xT[E,T], wqT/wkT[E,HL*HD] (pair-permuted), wvT[E,HL*HD],
    woT[HL*HD,E], cosF/sinF[P,T].  Output: outp[T,E] bf16 (partial, summed on host).
    """
    assert HD == P and E % P == 0 and T % P == 0 and T % CH == 0
    assert HL % 2 == 0 and HL * HD <= 512 and CH <= 512
    ET = E // P          # contraction tiles for the projections
    TC = T // CH         # t-chunks in phase A
    ST = T // P          # s-tiles
    D2 = HD // 2
    NPAIR = HL // 2
    CP = 1024            # attention t-chunk (2 PSUM banks wide)
    NCP = T // CP
    scale = 1.0 / math.sqrt(HD)

    nc = bacc.Bacc(None)
    xT = nc.dram_tensor("xT", [E, T], BF16, kind="ExternalInput")
    wqT = nc.dram_tensor("wqT", [E, HL * HD], BF16, kind="ExternalInput")
    wkT = nc.dram_tensor("wkT", [E, HL * HD], BF16, kind="ExternalInput")
    wvT = nc.dram_tensor("wvT", [E, HL * HD], BF16, kind="ExternalInput")
    woT = nc.dram_tensor("woT", [HL * HD, E], BF16, kind="ExternalInput")
    cosF = nc.dram_tensor("cosF", [P, T], F32, kind="ExternalInput")
    sinF = nc.dram_tensor("sinF", [P, T], F32, kind="ExternalInput")
    outp = nc.dram_tensor("outp", [T, E], BF16, kind="ExternalOutput")

    xT_t = xT.rearrange("(o p) t -> p o t", p=P)
    wq_t = wqT.rearrange("(o p) m -> p o m", p=P)
    wk_t = wkT.rearrange("(o p) m -> p o m", p=P)
    wv_t = wvT.rearrange("(o p) m -> p o m", p=P)
    wo_t = woT.rearrange("(h p) e -> p h e", p=P)

    with tile.TileContext(nc) as tc:
        with tc.tile_pool(name="keep", bufs=1) as keep:
            # persistent SBUF: Q^T/K^T per head [d, t], V [s, st, h*d], wo, attn
            qt_all = keep.tile([P, HL, T], BF16)
            kt_all = keep.tile([P, HL, T], BF16)
            v_all = keep.tile([P, ST, HL * HD], BF16)
            wo_sb = keep.tile([P, HL, E], BF16)
            attn0 = keep.tile([P, HL, CP], BF16)
            attn1 = keep.tile([P, HL, CP], BF16)
            attn_cp = [attn0, attn1]
            assert NCP == 2
            ones_sb = keep.tile([P, P], BF16)

            # ---------------- Phase A: QKV projections + RoPE ----------------
            with (
                tc.tile_pool(name="aw", bufs=1) as aw,
                tc.tile_pool(name="ax", bufs=2) as ax,
                tc.tile_pool(name="acs", bufs=2) as acs,
                tc.tile_pool(name="aps", bufs=2, space="PSUM") as aps,
                tc.tile_pool(name="arot", bufs=3) as arot,
            ):
                warm_f = aw.tile([P, CH], F32)
                nc.vector.memset(warm_f[:], 0.0)
                warm = aw.tile([P, CH], BF16)
                nc.vector.tensor_copy(warm[:], warm_f[:])
                ones_f32 = aw.tile([P, P], F32)
                nc.vector.memset(ones_f32[:], 1.0)
                nc.vector.tensor_copy(ones_sb[:], ones_f32[:])
                wq_sb = aw.tile([P, ET, HL * HD], BF16)
                wk_sb = aw.tile([P, ET, HL * HD], BF16)
                wv_sb = aw.tile([P, ET, HL * HD], BF16)
                x0_sb = ax.tile([P, ET, CH], BF16, tag="xc")
                # interleaved k-sliced loads: the k-th matmul of the first
                # accumulation only waits for its own slices.
                for k2 in range(0, ET, 2):
                    nc.sync.dma_start(wq_sb[:, k2:k2 + 2, :], wq_t[:, k2:k2 + 2, :])
                    nc.sync.dma_start(x0_sb[:, k2:k2 + 2, :], xT_t[:, k2:k2 + 2, 0:CH])
                for k2 in range(0, ET, 2):
                    nc.sync.dma_start(wk_sb[:, k2:k2 + 2, :], wk_t[:, k2:k2 + 2, :])
                for k2 in range(0, ET, 2):
                    nc.sync.dma_start(wv_sb[:, k2:k2 + 2, :], wv_t[:, k2:k2 + 2, :])
                for h in range(HL):
                    nc.gpsimd.dma_start(wo_sb[:, h, :], wo_t[:, h, :])

                with tc.tile_pool(name="wps", bufs=1, space="PSUM") as wps:
                    # p-state ramp + covers the initial weight/x DMA wait
                    wp = wps.tile([64, CH], F32)
                    for _ in range(13):
                        nc.tensor.matmul(wp[:], warm[:, 0:64], warm[:],
                                         start=True, stop=True)

                for c in range(TC):
                    csl = slice(c * CH, (c + 1) * CH)
                    if c == 0:
                        x_sb = x0_sb
                    else:
                        x_sb = ax.tile([P, ET, CH], BF16, tag="xc")
                        for k2 in range(0, ET, 4):
                            k3 = min(k2 + 4, ET)
                            nc.sync.dma_start(
                                x_sb[:, k2:k3, :], xT_t[:, k2:k3, csl])
                    cos_c = acs.tile([P, CH], F32, tag="cos")
                    sin_c = acs.tile([P, CH], F32, tag="sin")
                    nc.gpsimd.dma_start(cos_c[:], cosF[:, csl])
                    nc.gpsimd.dma_start(sin_c[:], sinF[:, csl])
                    for w_sb, dst in ((wq_sb, qt_all), (wk_sb, kt_all)):
                        for pr in range(NPAIR):
                            mA, mB = 2 * pr, 2 * pr + 1
                            psA = aps.tile([P, CH], F32, tag="psA")
                            psB = aps.tile([P, CH], F32, tag="psB")
                            for k in range(ET):
                                nc.tensor.matmul(
                                    psA[:], w_sb[:, k, mA * P:(mA + 1) * P],
                                    x_sb[:, k, :], start=(k == 0), stop=(k == ET - 1))
                            for k in range(ET):
                                nc.tensor.matmul(
                                    psB[:], w_sb[:, k, mB * P:(mB + 1) * P],
                                    x_sb[:, k, :], start=(k == 0), stop=(k == ET - 1))
                            # rope: psA rows = [hA even-dims | hB even-dims],
                            # psB rows = [hA odd | hB odd]
                            t1 = arot.tile([P, CH], F32, tag="t1")
                            t2 = arot.tile([P, CH], F32, tag="t2")
                            rA = arot.tile([P, CH], BF16, tag="rA")
                            nc.vector.tensor_tensor(t1[:], psA[:], cos_c[:], MUL)
                            nc.vector.tensor_tensor(t2[:], psB[:], sin_c[:], MUL)
                            nc.vector.tensor_tensor(rA[:], t1[:], t2[:], SUB)
                            t3 = arot.tile([P, CH], F32, tag="t1")
                            t4 = arot.tile([P, CH], F32, tag="t2")
                            rB = arot.tile([P, CH], BF16, tag="rA")
                            nc.vector.tensor_tensor(t3[:], psA[:], sin_c[:], MUL)
                            nc.vector.tensor_tensor(t4[:], psB[:], cos_c[:], MUL)
                            nc.vector.tensor_tensor(rB[:], t3[:], t4[:], ADD)
                            ha, hb = 2 * pr, 2 * pr + 1
                            # quadrant placement (partition shifts via DMA):
                            nc.gpsimd.dma_start(dst[0:D2, ha, csl], rA[0:D2, :])
                            nc.gpsimd.dma_start(dst[0:D2, hb, csl], rA[D2:P, :])
                            nc.gpsimd.dma_start(dst[D2:P, ha, csl], rB[0:D2, :])
                            nc.gpsimd.dma_start(dst[D2:P, hb, csl], rB[D2:P, :])
                    for st in range(CH // P):
                        psV = aps.tile([P, HL * HD], F32, tag="psV")
                        for k in range(ET):
                            nc.tensor.matmul(
                                psV[:], x_sb[:, k, st * P:(st + 1) * P],
                                wv_sb[:, k, :], start=(k == 0), stop=(k == ET - 1))
                        nc.scalar.copy(v_all[:, c * (CH // P) + st, :], psV[:])

            # ---------------- Phase B: attention + interleaved out-proj ----------------
            # C-groups: out[t_tile, oc] = sum_h attn[d, t_tile]^T @ wo[d, oc].
            # Output rows are batched: 4 oc-groups fill one [P, E] bf16 row
            # tile, then a single wide DMA (4KB lines) writes the row.
            c_rows = {}

            def emit_c_group(cp, tt, oc, ops_pool, row_pool, idx):
                tsl = slice(cp * CP + tt * P, cp * CP + (tt + 1) * P)
                osl = slice(oc * CH, (oc + 1) * CH)
                ops = ops_pool.tile([P, CP], F32, tag="sps")
                for hh in range(HL):
                    nc.tensor.matmul(
                        ops[:, 0:CH], attn_cp[cp][:, hh, tt * P:(tt + 1) * P],
                        wo_sb[:, hh, osl], start=(hh == 0), stop=(hh == HL - 1))
                if oc == 0:
                    c_rows[(cp, tt)] = row_pool.tile(
                        [P, E], BF16, tag="orow", name=f"orow_{cp}_{tt}")
                orow = c_rows[(cp, tt)]
                if idx % 2 == 0:
                    nc.vector.tensor_copy(orow[:, osl], ops[:, 0:CH])
                else:
                    nc.scalar.copy(orow[:, osl], ops[:, 0:CH])
                if oc == E // CH - 1:
                    if idx < 24:
                        eng = nc.sync if (tt % 2 == 0) else nc.gpsimd
                    else:  # tail: spread the big row writes over 3 queues
                        eng = (nc.sync, nc.gpsimd,
                               nc.scalar)[(cp * 8 + tt) % 3]
                    eng.dma_start(outp[tsl, :], orow[:])
                    del c_rows[(cp, tt)]

            c_groups = [(cp, tt, oc) for cp in range(NCP)
                        for tt in range(CP // P) for oc in range(E // CH)]
            c_next = 0
            with (
                tc.tile_pool(name="bsp", bufs=2, space="PSUM") as bsp,
                tc.tile_pool(name="bpv", bufs=1, space="PSUM") as bpv,
                tc.tile_pool(name="brsp", bufs=1, space="PSUM") as brsp,
                tc.tile_pool(name="bpt", bufs=6) as bpt,
                tc.tile_pool(name="bpair", bufs=4) as bpair,
                tc.tile_pool(name="brc", bufs=2) as brc,
                tc.tile_pool(name="bco", bufs=3) as bco,
            ):
                def emit_qk(cp, h, st):
                    """QK pair + exp for one s-tile; returns the exp tile."""
                    sps = bsp.tile([P, CP], F32, tag="sps",
                                   name=f"sps_{cp}_{h}_{st}")
                    ksl = kt_all[:, h, st * P:(st + 1) * P]
                    nc.tensor.matmul(
                        sps[:, 0:CH], ksl,
                        qt_all[:, h, cp * CP:cp * CP + CH],
                        start=True, stop=True)
                    nc.tensor.matmul(
                        sps[:, CH:CP], ksl,
                        qt_all[:, h, cp * CP + CH:(cp + 1) * CP],
                        start=True, stop=True)
                    pt = bpt.tile([P, CP], BF16, tag="pt",
                                  name=f"pt_{cp}_{h}_{st}")
                    nc.scalar.activation(
                        pt[:], sps[:], mybir.ActivationFunctionType.Exp,
                        scale=scale)
                    return pt

                carry_pt = None
                for cp in range(NCP):
                    for h in range(HL):
                        hsl = slice(h * HD, (h + 1) * HD)
                        pv = bpv.tile([P, CP], F32)
                        rs = brsp.tile([P, CP], F32)
                        prev_pt = None
                        pairs, quads = [], []
                        rs_pending = []  # (piece, ready_st); emit 2 sts later
                        rs_emitted = 0

                        def emit_rs(piece, is_last):
                            nonlocal rs_emitted
                            nc.tensor.matmul(
                                rs[:, 0:CH], ones_sb[:], piece[:, 0:CH],
                                start=(rs_emitted == 0), stop=is_last)
                            nc.tensor.matmul(
                                rs[:, CH:CP], ones_sb[:], piece[:, CH:CP],
                                start=(rs_emitted == 0), stop=is_last)
                            rs_emitted += 1

                        if carry_pt is None:
                            carry_pt = emit_qk(cp, h, 0)
                        pt_cur = carry_pt
                        carry_pt = None
                        for st in range(ST):
                            # QK/exp run one s-tile ahead of PV so the PE
                            # never waits on the activation at steady state
                            pt_next = emit_qk(cp, h, st + 1) if st + 1 < ST \
                                else None
                            pt = pt_cur
                            vsl = v_all[:, st, hsl]
                            nc.tensor.matmul(
                                pv[:, 0:CH], vsl, pt[:, 0:CH],
                                start=(st == 0), stop=(st == ST - 1))
                            nc.tensor.matmul(
                                pv[:, CH:CP], vsl, pt[:, CH:CP],
                                start=(st == 0), stop=(st == ST - 1))
                            pt_cur = pt_next
                            while rs_pending and rs_pending[0][1] + 2 <= st:
                                emit_rs(rs_pending.pop(0)[0], False)
                            # Graded DVE reduction: sts 0-7 reduce to an oct,
                            # 8-11 to a quad, 12-13 / 14-15 stay pairs.  Each
                            # finished piece feeds one ones-matmul, so the
                            # last row-sum matmul only trails the final
                            # pair-add (short chain), yet rs needs just 4
                            # accumulation steps per head.
                            if st % 2 == 0:
                                prev_pt = pt
                                continue
                            p2 = bpair.tile([P, CP], BF16, tag="p2", bufs=4)
                            nc.vector.tensor_tensor(
                                p2[:], prev_pt[:], pt[:], ADD)
                            piece = None
                            if st in (1, 5, 9):
                                pairs.append(p2)
                            elif st in (3, 7, 11):
                                q4 = bpair.tile([P, CP], BF16, tag="q4", bufs=3)
                                nc.vector.tensor_tensor(
                                    q4[:], pairs.pop()[:], p2[:], ADD)
                                if st == 11:
                                    piece = q4   # sts 8-11 reduced to a quad
                                else:
                                    quads.append(q4)
                                if st == 7:
                                    q8 = bpair.tile([P, CP], BF16, tag="q8",
                                                    bufs=2)
                                    nc.vector.tensor_tensor(
                                        q8[:], quads[0][:], quads[1][:], ADD)
                                    quads = []
                                    piece = q8   # sts 0-7 reduced to one oct
                            else:
                                piece = p2       # sts 12-13, 14-15: raw pair
                            if piece is not None:
                                rs_pending.append((piece, st))
                        # drain pv to SBUF immediately (releases its PSUM banks
                        # for the next head); normalize from the copy once the
                        # reciprocal lands — off the critical path.
                        attn_un = brc.tile([P, CP], BF16, tag="un")
                        nc.vector.tensor_copy(attn_un[:], pv[:])
                        # prefetch the next head's first QK/exp across the
                        # boundary so its PV never waits on the activation
                        if (cp, h) != (NCP - 1, HL - 1):
                            ncp_, nh_ = (cp, h + 1) if h + 1 < HL else (cp + 1, 0)
                            carry_pt = emit_qk(ncp_, nh_, 0)
                        while rs_pending:
                            piece, _ = rs_pending.pop(0)
                            emit_rs(piece, not rs_pending)
                        # cover the recip/normalize latency with out-proj work
                        if cp > 0:
                            for _ in range(3):
                                gcp, gtt, goc = c_groups[c_next]
                                emit_c_group(gcp, gtt, goc, bsp, bco, c_next)
                                c_next += 1
                        rec = brc.tile([P, CP], F32, tag="rec")
                        scr8 = brc.tile([P, CP], F32, tag="scr")
                        nc.vector.reciprocal_approx_accurate(
                            out=rec[:], in_=rs[:], scratch=scr8[:])
                        nc.vector.tensor_tensor(
                            attn_cp[cp][:, h, :], attn_un[:], rec[:], MUL)
                        if cp > 0:
                            for _ in range(3):
                                gcp, gtt, goc = c_groups[c_next]
                                emit_c_group(gcp, gtt, goc, bsp, bco, c_next)
                                c_next += 1

            # ---------------- Phase C tail: remaining out-proj groups ----------------
            with (
                tc.tile_pool(name="cout", bufs=3) as cout,
                tc.tile_pool(name="cps", bufs=3, space="PSUM") as cps,
            ):
                while c_next < len(c_groups):
                    gcp, gtt, goc = c_groups[c_next]
                    emit_c_group(gcp, gtt, goc, cps, cout, c_next)
                    c_next += 1

    nc.finalize()
    return nc


# ---------------------------------------------------------------------------
# Host-side wrapper
# ---------------------------------------------------------------------------

_B, _T, _EMB = 2, 2048, 2048
_HQ, _HD = 16, 128
_GROUPS = 4                      # head groups; 2 batches x 4 groups = 8 cores
_HL = _HQ // _GROUPS             # 4 local heads per core

_nc_cache = {}


def _get_nc():
    key = (_T, _EMB, _HL, _HD)
    if key not in _nc_cache:
        _nc_cache[key] = build_attention_nc(_T, _EMB, _HL, _HD, CH=512)
    return _nc_cache[key]


def _prep_core_inputs(x, wq, wk, wv, wo, fc, fs):
    """Per-core input dicts for 8 cores (core = 4*batch + group)."""
    import ml_dtypes
    bf16 = ml_dtypes.bfloat16
    # RoPE pair-permutation within each head: [even dims, odd dims]
    perm = np.concatenate([np.arange(0, _HD, 2), np.arange(1, _HD, 2)])
    cosF = np.ascontiguousarray(np.tile(fc.T, (2, 1)), dtype=np.float32)  # [128,T]
    sinF = np.ascontiguousarray(np.tile(fs.T, (2, 1)), dtype=np.float32)

    xT = [np.ascontiguousarray(x[b].T).astype(bf16) for b in range(_B)]

    in_maps = []
    for b in range(_B):
        for g in range(_GROUPS):
            heads = [g * _HL + h for h in range(_HL)]
            # device q/k row order: per pair (h0,h1): [h0_e, h1_e], [h0_o, h1_o]
            rows = []
            for pr in range(_HL // 2):
                h0, h1 = heads[2 * pr], heads[2 * pr + 1]
                for half in (perm[:64], perm[64:]):
                    rows.append(h0 * _HD + half)
                    rows.append(h1 * _HD + half)
            rows = np.concatenate(rows)
            vrows = np.concatenate([np.arange(h * _HD, (h + 1) * _HD) for h in heads])
            in_maps.append({
                "xT": xT[b],
                "wqT": np.ascontiguousarray(wq[rows].T).astype(bf16),
                "wkT": np.ascontiguousarray(wk[rows].T).astype(bf16),
                "wvT": np.ascontiguousarray(wv[vrows].T).astype(bf16),
                "woT": np.ascontiguousarray(wo[:, vrows].T).astype(bf16),
                "cosF": cosF,
                "sinF": sinF,
            })
    return in_maps


def kernel(**inputs):
    x = np.asarray(inputs["x"], dtype=np.float32)
    wq = np.asarray(inputs["wq"], dtype=np.float32)
    wk = np.asarray(inputs["wk"], dtype=np.float32)
    wv = np.asarray(inputs["wv"], dtype=np.float32)
    wo = np.asarray(inputs["wo"], dtype=np.float32)
    fc = np.asarray(inputs["freqs_cos"], dtype=np.float32)
    fs = np.asarray(inputs["freqs_sin"], dtype=np.float32)
    # start_pos is 0 (cache region [0, T) is fully overwritten before the read,
    # so k_cache/v_cache never contribute to the output).

    nc = _get_nc()
    in_maps = _prep_core_inputs(x, wq, wk, wv, wo, fc, fs)
    res = run_bass_kernel_spmd(nc, in_maps, core_ids=list(range(8)))

    out = np.empty((_B, _T, _EMB), dtype=np.float32)
    for b in range(_B):
        acc = np.zeros((_T, _EMB), dtype=np.float32)
        for g in range(_GROUPS):
            acc += res.results[4 * b + g]["outp"].astype(np.float32)
        out[b] = acc
    return out
